# revision 10
# baseline (speedup 1.0000x reference)
"""AssociativeEmbeddingLoss on 8 TRN2 NeuronCores, v8.

Reference, per image b (C=1, G=128 boxes):
    tl[g] = pred[b, 0, ty[g], tx[g]],  br[g] = target[b, 0, by[g], bx[g]]
    me = (tl + br) / 2
    pull_b = sum((tl-br)^2) / (2N)
    push_b = sum_{i != j} relu(1 - |me_i - me_j|) / (N*(N-1))
    out = (0.25 * sum_b pull_b, 0.25 * sum_b push_b)

Data-parallel over batch, 8 images per core (2048 scattered scalars).

Gather-architecture facts (measured on HW this session):
- INDIRECT1D consumes ONE offset per out-partition and streams
  out-free-size consecutive elements per descriptor; a [128,16] out
  with [128,16] offsets silently uses only offs[:,0] (128 descs of
  64B) and permutes the data. >128 scattered scalars per instruction
  is NOT expressible (3D out APs are rejected/garbage).
- INDIRECT1D costs ~994ns fixed + ~sub-ns/desc at 128 descs (2-desc
  warm-up and 128-desc gather both ~1.1us of Q7).
- DMAGatherAnt costs ~10.3ns/idx linear (256 idxs = 2.64us measured),
  and only moves 256B-aligned blocks -- no better.
So 16 x [128,1] INDIRECT1D at ~1.1-1.4us cadence is the floor for the
gather stream; everything else hides behind it.

v7 vs v4 (40.9us):
- offsets are precomputed on host (pure index arithmetic for the
  chosen shard): flat int32 indices into the per-core concat(pred,
  target) buffer, [128(g), 16(m=2b+tb)]. One HWDGE load replaces the
  on-device DVE offset math; the first gather only waits for that DMA.
- consts (bf16 identity, bf16 ones row, f32 ones col) come from DRAM
  pre-typed; no ACT conversion traffic.
- per-image push compute (v4-proven): transpose -> merow copy ->
  K=1 bf16 ones x merow matmul -> ACT Abs(0.5x, bias=-0.5me) in PSUM
  -> DVE min+accum lagging one image. ad stays in PSUM: Q7 descriptor
  generation contends on SBUF ports with DVE during the whole stream.
- joint reduction: pull sq block and the 8 min-accum columns share one
  [128,16] fin tile; a single ones-matmul + two activation
  accumulators produce both scalars.
Each core returns [pull_partial, push_partial]; the host sums the 8
pairs (unshard).
"""

import numpy as np
import ml_dtypes

import concourse.bacc as bacc
import concourse.mybir as mybir
import concourse.tile as tile
from concourse.bass import IndirectOffsetOnAxis
from concourse.bass_utils import run_bass_kernel_spmd

B, C, H, W = 64, 1, 512, 512
G = 128                 # boxes per image; N = G*C = 128
N = G * C
NCORES = 8
BP = B // NCORES        # images per core
NPIX = BP * H * W
M = 2 * BP              # gather columns: m = 2b + tb
PULL_W, PUSH_W = 0.25, 0.25

F32 = mybir.dt.float32
BF16 = mybir.dt.bfloat16
I32 = mybir.dt.int32
AF = mybir.ActivationFunctionType
ALU = mybir.AluOpType

C_PULL = PULL_W / (2.0 * N)
C_PUSH = PUSH_W / (N * (N - 1))


def _build_nc():
    nc = bacc.Bacc(
        "TRN2",
        target_bir_lowering=False,
        debug=False,
        enable_asserts=False,
        num_devices=NCORES,
        num_swdge_queues=2,
    )
    data = nc.dram_tensor("data", [2 * NPIX, 1], F32, kind="ExternalInput")
    offs = nc.dram_tensor("offs", [G, M], I32, kind="ExternalInput")
    onesd = nc.dram_tensor("onesd", [G, 1], F32, kind="ExternalInput")
    identd = nc.dram_tensor("identd", [G, G], BF16, kind="ExternalInput")
    rowsd = nc.dram_tensor("rowsd", [1, G], BF16, kind="ExternalInput")
    out = nc.dram_tensor("out", [1, 2], F32, kind="ExternalOutput")

    with tile.TileContext(nc) as tc:
        _kernel_body(nc, tc, data, offs, onesd, identd, rowsd, out)
    nc.compile()
    return nc


def _kernel_body(nc, tc, data, offs, onesd, identd, rowsd, out):
    with (
        tc.tile_pool(name="sb", bufs=1) as sb,
        tc.tile_pool(name="ps", bufs=1, space="PSUM") as ps,
        tc.tile_pool(name="psr", bufs=2, space="PSUM") as psr,
    ):
        # ---- loads; the first gather is gated only by the offs DMA ----
        off = sb.tile([G, M], I32, tag="off")
        nc.sync.dma_start(out=off[:], in_=offs.ap())
        ident16 = sb.tile([G, G], BF16, tag="ident16")
        nc.scalar.dma_start(out=ident16[:], in_=identd.ap())
        ones16 = sb.tile([1, G], BF16, tag="ones16")
        nc.scalar.dma_start(out=ones16[:], in_=rowsd.ap())
        onescol = sb.tile([G, 1], F32, tag="onescol")
        nc.scalar.dma_start(out=onescol[:], in_=onesd.ap())

        # preload the Abs activation table while DMAs are in flight
        scrd = sb.tile([1, 1], F32, tag="scrd")
        nc.scalar.activation(out=scrd[:], in_=onescol[0:1, 0:1], func=AF.Abs)

        # warm the INDIRECT1D path on the idle Q7 during the preamble so
        # the first real gather skips first-use setup (~0.5us)
        zoff = sb.tile([2, 1], I32, tag="zoff")
        nc.gpsimd.memset(zoff[:], 0)
        wscrap = sb.tile([2, 2], F32, tag="wscrap")
        for q in range(2):
            wi = nc.gpsimd.indirect_dma_start(
                out=wscrap[:, q : q + 1], out_offset=None, in_=data.ap(),
                in_offset=IndirectOffsetOnAxis(ap=zoff[:], axis=0),
            )
            if q:
                wi.ins.queue = "qPoolDynamic1"

        # ---- 16 gathers streaming on gpsimd ----
        dcol = sb.tile([G, M], F32, tag="dcol")
        for m in range(M):
            gi = nc.gpsimd.indirect_dma_start(
                out=dcol[:, m : m + 1], out_offset=None, in_=data.ap(),
                in_offset=IndirectOffsetOnAxis(ap=off[:, m : m + 1], axis=0),
            )
            if m % 2:
                # alternate SWDGE rings; the two queues' descriptor
                # generation can overlap
                gi.ins.queue = "qPoolDynamic1"

        dv = dcol[:].rearrange("g (b t) -> g b t", b=BP, t=2)
        me = sb.tile([G, BP], BF16, tag="me")
        negme = sb.tile([G, BP], F32, tag="negme")
        fin = sb.tile([G, M], F32, tag="fin")   # cols 0:8 sq, 8:16 min

        def push_image(b):
            bs = slice(b, b + 1)
            nc.vector.tensor_tensor(out=me[:, bs], in0=dv[:, b, 0:1],
                                    in1=dv[:, b, 1:2], op=ALU.add)
            nc.vector.tensor_scalar(out=negme[:, bs], in0=me[:, bs],
                                    scalar1=-0.5, scalar2=None, op0=ALU.mult)
            rowp = psr.tile([1, G], BF16, tag="rowp")
            nc.tensor.transpose(out=rowp[:], in_=me[:, bs], identity=ident16[:])
            merow = sb.tile([1, G], BF16, tag=f"merow{b % 2}")
            nc.vector.tensor_copy(out=merow[:], in_=rowp[:])
            Rp = psr.tile([G, G], F32, tag="Rp")
            nc.tensor.matmul(out=Rp[:], lhsT=ones16[:], rhs=merow[:],
                             start=True, stop=True)
            # ad lives in PSUM: the Abs write and the min read stay off
            # the SBUF ports that Q7 descriptor generation contends on
            ad = psr.tile([G, G], F32, tag="ad")
            nc.scalar.activation(out=ad[:], in_=Rp[:], func=AF.Abs,
                                 bias=negme[:, bs], scale=0.5)
            return ad

        ads = [None, None]
        for b in range(BP):
            # lag the DVE min by one image so the vector queue never
            # stalls waiting on this image's ABS
            if b >= 1:
                pb = b - 1
                nc.vector.tensor_scalar(
                    out=ads[pb % 2][:], in0=ads[pb % 2][:], scalar1=1.0,
                    scalar2=0.0, op0=ALU.min, op1=ALU.add,
                    accum_out=fin[:, BP + pb : BP + pb + 1],
                )
            ads[b % 2] = push_image(b)

        # pull: bulk dsub/sq + the joint reduction run under the last ABS
        dsub = sb.tile([G, BP], F32, tag="dsub")
        nc.vector.tensor_tensor(out=dsub[:], in0=dv[:, :, 0], in1=dv[:, :, 1],
                                op=ALU.subtract)
        nc.vector.tensor_tensor(out=fin[:, 0:BP], in0=dsub[:], in1=dsub[:],
                                op=ALU.mult)
        nc.vector.tensor_scalar(
            out=ads[(BP - 1) % 2][:], in0=ads[(BP - 1) % 2][:], scalar1=1.0,
            scalar2=0.0, op0=ALU.min, op1=ALU.add,
            accum_out=fin[:, M - 1 : M],
        )

        # ---- joint reduction: one ones-matmul + two accum activations ----
        pg = ps.tile([1, M], F32, tag="pg")
        scr = sb.tile([1, M], F32, tag="scr")
        res = sb.tile([1, 2], F32, tag="res")
        nc.tensor.matmul(out=pg[:], lhsT=onescol[:], rhs=fin[:], start=True,
                         stop=True)
        nc.scalar.activation(out=scr[0:1, 0:BP], in_=pg[0:1, 0:BP], func=AF.Copy,
                             scale=C_PULL, accum_out=res[0:1, 0:1])
        nc.scalar.activation(out=scr[0:1, BP:M], in_=pg[0:1, BP:M],
                             func=AF.Copy, scale=-C_PUSH,
                             bias=float(N * (N - 1)) * C_PUSH,
                             accum_out=res[0:1, 1:2])
        nc.sync.dma_start(out=out.ap(), in_=res[:])


_NC_CACHE = None


def _get_nc():
    global _NC_CACHE
    if _NC_CACHE is None:
        _NC_CACHE = _build_nc()
    return _NC_CACHE


def _consts():
    ones = np.ones((G, 1), dtype=np.float32)
    ident = np.eye(G, dtype=np.float32).astype(ml_dtypes.bfloat16)
    onesrow = np.ones((1, G), dtype=ml_dtypes.bfloat16)
    return ones, ident, onesrow


def make_in_maps(pred, target, match):
    pred = np.asarray(pred, dtype=np.float32).reshape(B, H * W)
    target = np.asarray(target, dtype=np.float32).reshape(B, H * W)
    match = np.asarray(match).astype(np.int64)
    ones, ident, onesrow = _consts()
    HW = H * W
    in_maps = []
    for k in range(NCORES):
        sl = slice(k * BP, (k + 1) * BP)
        data = np.concatenate(
            [pred[sl].reshape(-1), target[sl].reshape(-1)]
        ).reshape(2 * NPIX, 1)
        m = match[sl]  # [BP, G, 2, 2]
        offs = np.empty((G, M), dtype=np.int32)
        for b in range(BP):
            offs[:, 2 * b] = b * HW + m[b, :, 0, 0] * W + m[b, :, 0, 1]
            offs[:, 2 * b + 1] = NPIX + b * HW + m[b, :, 1, 0] * W + m[b, :, 1, 1]
        in_maps.append({
            "data": data,
            "offs": offs,
            "onesd": ones,
            "identd": ident,
            "rowsd": onesrow,
        })
    return in_maps


def kernel(pred, target, match, _trace=False):
    nc = _get_nc()
    in_maps = make_in_maps(pred, target, match)
    res = run_bass_kernel_spmd(nc, in_maps, core_ids=list(range(NCORES)), trace=_trace)
    total = np.zeros((1, 2), dtype=np.float64)
    for r in res.results:
        total += r["out"].astype(np.float64)
    out = (np.float32(total[0, 0]), np.float32(total[0, 1]))
    if _trace:
        return out, res
    return out


# revision 12
# speedup vs baseline: 1.0194x; 1.0194x over previous
"""AssociativeEmbeddingLoss on 8 TRN2 NeuronCores, v7.

Reference, per image b (C=1, G=128 boxes):
    tl[g] = pred[b, 0, ty[g], tx[g]],  br[g] = target[b, 0, by[g], bx[g]]
    me = (tl + br) / 2
    pull_b = sum((tl-br)^2) / (2N)
    push_b = sum_{i != j} relu(1 - |me_i - me_j|) / (N*(N-1))
    out = (0.25 * sum_b pull_b, 0.25 * sum_b push_b)

Data-parallel over batch, 8 images per core (2048 scattered scalars).

Gather-architecture facts (measured on HW this session):
- INDIRECT1D consumes ONE offset per out-partition and streams
  out-free-size consecutive elements per descriptor; a [128,16] out
  with [128,16] offsets silently uses only offs[:,0] (128 descs of
  64B) and permutes the data. >128 scattered scalars per instruction
  is NOT expressible (3D out APs are rejected/garbage).
- INDIRECT1D costs ~994ns fixed + ~sub-ns/desc at 128 descs (2-desc
  warm-up and 128-desc gather both ~1.1us of Q7).
- DMAGatherAnt costs ~10.3ns/idx linear (256 idxs = 2.64us measured),
  and only moves 256B-aligned blocks -- no better.
So 16 x [128,1] INDIRECT1D at ~1.1-1.4us cadence is the floor for the
gather stream; everything else hides behind it.

v7 vs v4 (40.9us):
- offsets are precomputed on host (pure index arithmetic for the
  chosen shard): flat int32 indices into the per-core concat(pred,
  target) buffer, [128(g), 16(m=2b+tb)]. One HWDGE load replaces the
  on-device DVE offset math; the first gather only waits for that DMA.
- consts (bf16 identity, bf16 ones row, f32 ones col) come from DRAM
  pre-typed; no ACT conversion traffic.
- per-image push compute (v4-proven): transpose -> merow copy ->
  K=1 bf16 ones x merow matmul -> ACT Abs(0.5x, bias=-0.5me) in PSUM
  -> DVE min+accum lagging one image. ad stays in PSUM: Q7 descriptor
  generation contends on SBUF ports with DVE during the whole stream.
- joint reduction: pull sq block and the 8 min-accum columns share one
  [128,16] fin tile; a single ones-matmul + two activation
  accumulators produce both scalars.
Each core returns [pull_partial, push_partial]; the host sums the 8
pairs (unshard).
"""

import numpy as np
import ml_dtypes

import concourse.bacc as bacc
import concourse.mybir as mybir
import concourse.tile as tile
from concourse.bass import IndirectOffsetOnAxis
from concourse.bass_utils import run_bass_kernel_spmd

B, C, H, W = 64, 1, 512, 512
G = 128                 # boxes per image; N = G*C = 128
N = G * C
NCORES = 8
BP = B // NCORES        # images per core
NPIX = BP * H * W
M = 2 * BP              # gather columns: m = 2b + tb
PULL_W, PUSH_W = 0.25, 0.25

F32 = mybir.dt.float32
BF16 = mybir.dt.bfloat16
I32 = mybir.dt.int32
AF = mybir.ActivationFunctionType
ALU = mybir.AluOpType

C_PULL = PULL_W / (2.0 * N)
C_PUSH = PUSH_W / (N * (N - 1))


def _build_nc():
    nc = bacc.Bacc(
        "TRN2",
        target_bir_lowering=False,
        debug=False,
        enable_asserts=False,
        num_devices=NCORES,
    )
    data = nc.dram_tensor("data", [2 * NPIX, 1], F32, kind="ExternalInput")
    offs = nc.dram_tensor("offs", [G, M], I32, kind="ExternalInput")
    onesd = nc.dram_tensor("onesd", [G, 1], F32, kind="ExternalInput")
    identd = nc.dram_tensor("identd", [G, G], BF16, kind="ExternalInput")
    rowsd = nc.dram_tensor("rowsd", [1, G], BF16, kind="ExternalInput")
    out = nc.dram_tensor("out", [1, M], F32, kind="ExternalOutput")

    with tile.TileContext(nc) as tc:
        _kernel_body(nc, tc, data, offs, onesd, identd, rowsd, out)
    nc.compile()
    return nc


def _kernel_body(nc, tc, data, offs, onesd, identd, rowsd, out):
    with (
        tc.tile_pool(name="sb", bufs=1) as sb,
        tc.tile_pool(name="ps", bufs=1, space="PSUM") as ps,
        tc.tile_pool(name="psr", bufs=2, space="PSUM") as psr,
    ):
        # ---- loads; the first gather is gated only by the offs DMA,
        # issued on the gpsimd queue itself (earliest possible issue,
        # and it warms the SWDGE path; first-use INDIRECT1D setup is
        # only ~0.1us, not worth a dedicated warm-up)
        off = sb.tile([G, M], I32, tag="off")
        nc.gpsimd.dma_start(out=off[:], in_=offs.ap())
        ident16 = sb.tile([G, G], BF16, tag="ident16")
        nc.scalar.dma_start(out=ident16[:], in_=identd.ap())
        ones16 = sb.tile([1, G], BF16, tag="ones16")
        nc.scalar.dma_start(out=ones16[:], in_=rowsd.ap())
        onescol = sb.tile([G, 1], F32, tag="onescol")
        nc.scalar.dma_start(out=onescol[:], in_=onesd.ap())

        # preload the Abs activation table while DMAs are in flight
        scrd = sb.tile([1, 1], F32, tag="scrd")
        nc.scalar.activation(out=scrd[:], in_=onescol[0:1, 0:1], func=AF.Abs)

        # ---- 16 gathers streaming on gpsimd ----
        dcol = sb.tile([G, M], F32, tag="dcol")
        for m in range(M):
            nc.gpsimd.indirect_dma_start(
                out=dcol[:, m : m + 1], out_offset=None, in_=data.ap(),
                in_offset=IndirectOffsetOnAxis(ap=off[:, m : m + 1], axis=0),
            )

        dv = dcol[:].rearrange("g (b t) -> g b t", b=BP, t=2)
        me = sb.tile([G, BP], BF16, tag="me")
        negme = sb.tile([G, BP], F32, tag="negme")
        fin = sb.tile([G, M], F32, tag="fin")   # cols 0:8 sq, 8:16 min

        dsub = sb.tile([G, BP], F32, tag="dsub")

        def push_image(b):
            bs = slice(b, b + 1)
            nc.vector.tensor_tensor(out=me[:, bs], in0=dv[:, b, 0:1],
                                    in1=dv[:, b, 1:2], op=ALU.add)
            nc.vector.tensor_tensor(out=dsub[:, bs], in0=dv[:, b, 0:1],
                                    in1=dv[:, b, 1:2], op=ALU.subtract)
            nc.vector.tensor_tensor(out=fin[:, bs], in0=dsub[:, bs],
                                    in1=dsub[:, bs], op=ALU.mult)
            nc.vector.tensor_scalar(out=negme[:, bs], in0=me[:, bs],
                                    scalar1=-0.5, scalar2=None, op0=ALU.mult)
            rowp = psr.tile([1, G], BF16, tag="rowp")
            nc.tensor.transpose(out=rowp[:], in_=me[:, bs], identity=ident16[:])
            merow = sb.tile([1, G], BF16, tag=f"merow{b % 2}")
            nc.vector.tensor_copy(out=merow[:], in_=rowp[:])
            Rp = psr.tile([G, G], F32, tag="Rp")
            nc.tensor.matmul(out=Rp[:], lhsT=ones16[:], rhs=merow[:],
                             start=True, stop=True)
            # ad lives in PSUM: the Abs write and the min read stay off
            # the SBUF ports that Q7 descriptor generation contends on
            ad = psr.tile([G, G], F32, tag="ad")
            nc.scalar.activation(out=ad[:], in_=Rp[:], func=AF.Abs,
                                 bias=negme[:, bs], scale=0.5)
            return ad

        ads = [None, None]
        for b in range(BP):
            # lag the DVE min by one image so the vector queue never
            # stalls waiting on this image's ABS
            if b >= 1:
                pb = b - 1
                nc.vector.tensor_scalar(
                    out=ads[pb % 2][:], in0=ads[pb % 2][:], scalar1=1.0,
                    scalar2=0.0, op0=ALU.min, op1=ALU.add,
                    accum_out=fin[:, BP + pb : BP + pb + 1],
                )
            ads[b % 2] = push_image(b)

        nc.vector.tensor_scalar(
            out=ads[(BP - 1) % 2][:], in0=ads[(BP - 1) % 2][:], scalar1=1.0,
            scalar2=0.0, op0=ALU.min, op1=ALU.add,
            accum_out=fin[:, M - 1 : M],
        )

        # ---- joint reduction: one ones-matmul; the 16 column sums go
        # back as-is (the host all-reduce applies the two scalar affine
        # maps; cross-core reduction is host-side regardless) ----
        pg = ps.tile([1, M], F32, tag="pg")
        pgs = sb.tile([1, M], F32, tag="pgs")
        nc.tensor.matmul(out=pg[:], lhsT=onescol[:], rhs=fin[:], start=True,
                         stop=True)
        nc.vector.tensor_copy(out=pgs[:], in_=pg[:])
        nc.sync.dma_start(out=out.ap(), in_=pgs[:])


_NC_CACHE = None


def _get_nc():
    global _NC_CACHE
    if _NC_CACHE is None:
        _NC_CACHE = _build_nc()
    return _NC_CACHE


def _consts():
    ones = np.ones((G, 1), dtype=np.float32)
    ident = np.eye(G, dtype=np.float32).astype(ml_dtypes.bfloat16)
    onesrow = np.ones((1, G), dtype=ml_dtypes.bfloat16)
    return ones, ident, onesrow


def make_in_maps(pred, target, match):
    pred = np.asarray(pred, dtype=np.float32).reshape(B, H * W)
    target = np.asarray(target, dtype=np.float32).reshape(B, H * W)
    match = np.asarray(match).astype(np.int64)
    ones, ident, onesrow = _consts()
    HW = H * W
    in_maps = []
    for k in range(NCORES):
        sl = slice(k * BP, (k + 1) * BP)
        data = np.concatenate(
            [pred[sl].reshape(-1), target[sl].reshape(-1)]
        ).reshape(2 * NPIX, 1)
        m = match[sl]  # [BP, G, 2, 2]
        offs = np.empty((G, M), dtype=np.int32)
        for b in range(BP):
            offs[:, 2 * b] = b * HW + m[b, :, 0, 0] * W + m[b, :, 0, 1]
            offs[:, 2 * b + 1] = NPIX + b * HW + m[b, :, 1, 0] * W + m[b, :, 1, 1]
        in_maps.append({
            "data": data,
            "offs": offs,
            "onesd": ones,
            "identd": ident,
            "rowsd": onesrow,
        })
    return in_maps


def kernel(pred, target, match, _trace=False):
    nc = _get_nc()
    in_maps = make_in_maps(pred, target, match)
    res = run_bass_kernel_spmd(nc, in_maps, core_ids=list(range(NCORES)), trace=_trace)
    total = np.zeros((1, M), dtype=np.float64)
    for r in res.results:
        total += r["out"].astype(np.float64)
    pull = C_PULL * float(total[0, 0:BP].sum())
    push = NCORES * BP * N * (N - 1) * C_PUSH - C_PUSH * float(
        total[0, BP:M].sum())
    out = (np.float32(pull), np.float32(push))
    if _trace:
        return out, res
    return out


# revision 13
# speedup vs baseline: 1.0461x; 1.0262x over previous
"""AssociativeEmbeddingLoss on 8 TRN2 NeuronCores, v7.

Reference, per image b (C=1, G=128 boxes):
    tl[g] = pred[b, 0, ty[g], tx[g]],  br[g] = target[b, 0, by[g], bx[g]]
    me = (tl + br) / 2
    pull_b = sum((tl-br)^2) / (2N)
    push_b = sum_{i != j} relu(1 - |me_i - me_j|) / (N*(N-1))
    out = (0.25 * sum_b pull_b, 0.25 * sum_b push_b)

Data-parallel over batch, 8 images per core (2048 scattered scalars).

Gather-architecture facts (measured on HW this session):
- INDIRECT1D consumes ONE offset per out-partition and streams
  out-free-size consecutive elements per descriptor; a [128,16] out
  with [128,16] offsets silently uses only offs[:,0] (128 descs of
  64B) and permutes the data. >128 scattered scalars per instruction
  is NOT expressible (3D out APs are rejected/garbage).
- INDIRECT1D costs ~994ns fixed + ~sub-ns/desc at 128 descs (2-desc
  warm-up and 128-desc gather both ~1.1us of Q7).
- DMAGatherAnt costs ~10.3ns/idx linear (256 idxs = 2.64us measured),
  and only moves 256B-aligned blocks -- no better.
So 16 x [128,1] INDIRECT1D at ~1.1-1.4us cadence is the floor for the
gather stream; everything else hides behind it.

v7 vs v4 (40.9us):
- offsets are precomputed on host (pure index arithmetic for the
  chosen shard): flat int32 indices into the per-core concat(pred,
  target) buffer, [128(g), 16(m=2b+tb)]. One HWDGE load replaces the
  on-device DVE offset math; the first gather only waits for that DMA.
- consts (bf16 identity, bf16 ones row, f32 ones col) come from DRAM
  pre-typed; no ACT conversion traffic.
- per-image push compute (v4-proven): transpose -> merow copy ->
  K=1 bf16 ones x merow matmul -> ACT Abs(0.5x, bias=-0.5me) in PSUM
  -> DVE min+accum lagging one image. ad stays in PSUM: Q7 descriptor
  generation contends on SBUF ports with DVE during the whole stream.
- joint reduction: pull sq block and the 8 min-accum columns share one
  [128,16] fin tile; a single ones-matmul + two activation
  accumulators produce both scalars.
Each core returns [pull_partial, push_partial]; the host sums the 8
pairs (unshard).
"""

import numpy as np
import ml_dtypes

import concourse.bacc as bacc
import concourse.mybir as mybir
import concourse.tile as tile
from concourse.bass import IndirectOffsetOnAxis
from concourse.bass_utils import run_bass_kernel_spmd

B, C, H, W = 64, 1, 512, 512
G = 128                 # boxes per image; N = G*C = 128
N = G * C
NCORES = 8
BP = B // NCORES        # images per core
NPIX = BP * H * W
M = 2 * BP              # gather columns: m = 2b + tb
PULL_W, PUSH_W = 0.25, 0.25

F32 = mybir.dt.float32
BF16 = mybir.dt.bfloat16
I32 = mybir.dt.int32
AF = mybir.ActivationFunctionType
ALU = mybir.AluOpType

C_PULL = PULL_W / (2.0 * N)
C_PUSH = PUSH_W / (N * (N - 1))


def _build_nc():
    nc = bacc.Bacc(
        "TRN2",
        target_bir_lowering=False,
        debug=False,
        enable_asserts=False,
        num_devices=NCORES,
    )
    data = nc.dram_tensor("data", [2 * NPIX, 1], F32, kind="ExternalInput")
    offs = nc.dram_tensor("offs", [G, M], I32, kind="ExternalInput")
    onesd = nc.dram_tensor("onesd", [G, 1], F32, kind="ExternalInput")
    identd = nc.dram_tensor("identd", [G, G], BF16, kind="ExternalInput")
    rowsd = nc.dram_tensor("rowsd", [1, G], BF16, kind="ExternalInput")
    out = nc.dram_tensor("out", [G, M], F32, kind="ExternalOutput")

    with tile.TileContext(nc) as tc:
        _kernel_body(nc, tc, data, offs, onesd, identd, rowsd, out)
    nc.compile()
    return nc


def _kernel_body(nc, tc, data, offs, onesd, identd, rowsd, out):
    with (
        tc.tile_pool(name="sb", bufs=1) as sb,
        tc.tile_pool(name="ps", bufs=1, space="PSUM") as ps,
        tc.tile_pool(name="psr", bufs=2, space="PSUM") as psr,
    ):
        # ---- loads; the first gather is gated only by the offs DMA,
        # issued on the gpsimd queue itself (earliest possible issue,
        # and it warms the SWDGE path; first-use INDIRECT1D setup is
        # only ~0.1us, not worth a dedicated warm-up)
        off = sb.tile([G, M], I32, tag="off")
        nc.gpsimd.dma_start(out=off[:], in_=offs.ap())
        ident16 = sb.tile([G, G], BF16, tag="ident16")
        nc.scalar.dma_start(out=ident16[:], in_=identd.ap())
        ones16 = sb.tile([1, G], BF16, tag="ones16")
        nc.scalar.dma_start(out=ones16[:], in_=rowsd.ap())
        onescol = sb.tile([G, 1], F32, tag="onescol")
        nc.scalar.dma_start(out=onescol[:], in_=onesd.ap())

        # preload the Abs activation table while DMAs are in flight
        scrd = sb.tile([1, 1], F32, tag="scrd")
        nc.scalar.activation(out=scrd[:], in_=onescol[0:1, 0:1], func=AF.Abs)

        # ---- 16 gathers streaming on gpsimd ----
        dcol = sb.tile([G, M], F32, tag="dcol")
        for m in range(M):
            nc.gpsimd.indirect_dma_start(
                out=dcol[:, m : m + 1], out_offset=None, in_=data.ap(),
                in_offset=IndirectOffsetOnAxis(ap=off[:, m : m + 1], axis=0),
            )

        dv = dcol[:].rearrange("g (b t) -> g b t", b=BP, t=2)
        me = sb.tile([G, BP], BF16, tag="me")
        negme = sb.tile([G, BP], F32, tag="negme")
        fin = sb.tile([G, M], F32, tag="fin")   # cols 0:8 sq, 8:16 min

        dsub = sb.tile([G, BP], F32, tag="dsub")

        def push_image(b):
            bs = slice(b, b + 1)
            nc.vector.tensor_tensor(out=me[:, bs], in0=dv[:, b, 0:1],
                                    in1=dv[:, b, 1:2], op=ALU.add)
            nc.vector.tensor_tensor(out=dsub[:, bs], in0=dv[:, b, 0:1],
                                    in1=dv[:, b, 1:2], op=ALU.subtract)
            nc.vector.tensor_tensor(out=fin[:, bs], in0=dsub[:, bs],
                                    in1=dsub[:, bs], op=ALU.mult)
            nc.vector.tensor_scalar(out=negme[:, bs], in0=me[:, bs],
                                    scalar1=-0.5, scalar2=None, op0=ALU.mult)
            rowp = psr.tile([1, G], BF16, tag="rowp")
            nc.tensor.transpose(out=rowp[:], in_=me[:, bs], identity=ident16[:])
            merow = sb.tile([1, G], BF16, tag=f"merow{b % 2}")
            nc.vector.tensor_copy(out=merow[:], in_=rowp[:])
            Rp = psr.tile([G, G], F32, tag="Rp")
            nc.tensor.matmul(out=Rp[:], lhsT=ones16[:], rhs=merow[:],
                             start=True, stop=True)
            # ad lives in PSUM: the Abs write and the min read stay off
            # the SBUF ports that Q7 descriptor generation contends on
            ad = psr.tile([G, G], F32, tag="ad")
            nc.scalar.activation(out=ad[:], in_=Rp[:], func=AF.Abs,
                                 bias=negme[:, bs], scale=0.5)
            return ad

        ads = [None, None]
        for b in range(BP):
            # lag the DVE min by one image so the vector queue never
            # stalls waiting on this image's ABS
            if b >= 1:
                pb = b - 1
                nc.vector.tensor_scalar(
                    out=ads[pb % 2][:], in0=ads[pb % 2][:], scalar1=1.0,
                    scalar2=0.0, op0=ALU.min, op1=ALU.add,
                    accum_out=fin[:, BP + pb : BP + pb + 1],
                )
            ads[b % 2] = push_image(b)

        nc.vector.tensor_scalar(
            out=ads[(BP - 1) % 2][:], in0=ads[(BP - 1) % 2][:], scalar1=1.0,
            scalar2=0.0, op0=ALU.min, op1=ALU.add,
            accum_out=fin[:, M - 1 : M],
        )

        # ---- ship fin [128,16] back; the host all-reduce does the
        # column sums + the two scalar affine maps (cross-core
        # reduction is host-side regardless) ----
        nc.sync.dma_start(out=out.ap(), in_=fin[:])


_NC_CACHE = None


def _get_nc():
    global _NC_CACHE
    if _NC_CACHE is None:
        _NC_CACHE = _build_nc()
    return _NC_CACHE


def _consts():
    ones = np.ones((G, 1), dtype=np.float32)
    ident = np.eye(G, dtype=np.float32).astype(ml_dtypes.bfloat16)
    onesrow = np.ones((1, G), dtype=ml_dtypes.bfloat16)
    return ones, ident, onesrow


def make_in_maps(pred, target, match):
    pred = np.asarray(pred, dtype=np.float32).reshape(B, H * W)
    target = np.asarray(target, dtype=np.float32).reshape(B, H * W)
    match = np.asarray(match).astype(np.int64)
    ones, ident, onesrow = _consts()
    HW = H * W
    in_maps = []
    for k in range(NCORES):
        sl = slice(k * BP, (k + 1) * BP)
        data = np.concatenate(
            [pred[sl].reshape(-1), target[sl].reshape(-1)]
        ).reshape(2 * NPIX, 1)
        m = match[sl]  # [BP, G, 2, 2]
        offs = np.empty((G, M), dtype=np.int32)
        for b in range(BP):
            offs[:, 2 * b] = b * HW + m[b, :, 0, 0] * W + m[b, :, 0, 1]
            offs[:, 2 * b + 1] = NPIX + b * HW + m[b, :, 1, 0] * W + m[b, :, 1, 1]
        in_maps.append({
            "data": data,
            "offs": offs,
            "onesd": ones,
            "identd": ident,
            "rowsd": onesrow,
        })
    return in_maps


def kernel(pred, target, match, _trace=False):
    nc = _get_nc()
    in_maps = make_in_maps(pred, target, match)
    res = run_bass_kernel_spmd(nc, in_maps, core_ids=list(range(NCORES)), trace=_trace)
    total = np.zeros((M,), dtype=np.float64)
    for r in res.results:
        total += r["out"].astype(np.float64).sum(axis=0)
    pull = C_PULL * float(total[0:BP].sum())
    push = NCORES * BP * N * (N - 1) * C_PUSH - C_PUSH * float(
        total[BP:M].sum())
    out = (np.float32(pull), np.float32(push))
    if _trace:
        return out, res
    return out


# revision 14
# speedup vs baseline: 1.0570x; 1.0104x over previous
"""AssociativeEmbeddingLoss on 8 TRN2 NeuronCores, v7.

Reference, per image b (C=1, G=128 boxes):
    tl[g] = pred[b, 0, ty[g], tx[g]],  br[g] = target[b, 0, by[g], bx[g]]
    me = (tl + br) / 2
    pull_b = sum((tl-br)^2) / (2N)
    push_b = sum_{i != j} relu(1 - |me_i - me_j|) / (N*(N-1))
    out = (0.25 * sum_b pull_b, 0.25 * sum_b push_b)

Data-parallel over batch, 8 images per core (2048 scattered scalars).

Gather-architecture facts (measured on HW this session):
- INDIRECT1D consumes ONE offset per out-partition and streams
  out-free-size consecutive elements per descriptor; a [128,16] out
  with [128,16] offsets silently uses only offs[:,0] (128 descs of
  64B) and permutes the data. >128 scattered scalars per instruction
  is NOT expressible (3D out APs are rejected/garbage).
- INDIRECT1D costs ~994ns fixed + ~sub-ns/desc at 128 descs (2-desc
  warm-up and 128-desc gather both ~1.1us of Q7).
- DMAGatherAnt costs ~10.3ns/idx linear (256 idxs = 2.64us measured),
  and only moves 256B-aligned blocks -- no better.
So 16 x [128,1] INDIRECT1D at ~1.1-1.4us cadence is the floor for the
gather stream; everything else hides behind it.

v7 vs v4 (40.9us):
- offsets are precomputed on host (pure index arithmetic for the
  chosen shard): flat int32 indices into the per-core concat(pred,
  target) buffer, [128(g), 16(m=2b+tb)]. One HWDGE load replaces the
  on-device DVE offset math; the first gather only waits for that DMA.
- consts (bf16 identity, bf16 ones row, f32 ones col) come from DRAM
  pre-typed; no ACT conversion traffic.
- per-image push compute (v4-proven): transpose -> merow copy ->
  K=1 bf16 ones x merow matmul -> ACT Abs(0.5x, bias=-0.5me) in PSUM
  -> DVE min+accum lagging one image. ad stays in PSUM: Q7 descriptor
  generation contends on SBUF ports with DVE during the whole stream.
- joint reduction: pull sq block and the 8 min-accum columns share one
  [128,16] fin tile; a single ones-matmul + two activation
  accumulators produce both scalars.
Each core returns [pull_partial, push_partial]; the host sums the 8
pairs (unshard).
"""

import numpy as np
import ml_dtypes

import concourse.bacc as bacc
import concourse.mybir as mybir
import concourse.tile as tile
from concourse.bass import IndirectOffsetOnAxis
from concourse.bass_utils import run_bass_kernel_spmd

B, C, H, W = 64, 1, 512, 512
G = 128                 # boxes per image; N = G*C = 128
N = G * C
NCORES = 8
BP = B // NCORES        # images per core
NPIX = BP * H * W
M = 2 * BP              # gather columns: m = 2b + tb
PULL_W, PUSH_W = 0.25, 0.25

F32 = mybir.dt.float32
BF16 = mybir.dt.bfloat16
I32 = mybir.dt.int32
AF = mybir.ActivationFunctionType
ALU = mybir.AluOpType

C_PULL = PULL_W / (2.0 * N)
C_PUSH = PUSH_W / (N * (N - 1))


def _build_nc():
    nc = bacc.Bacc(
        "TRN2",
        target_bir_lowering=False,
        debug=False,
        enable_asserts=False,
        num_devices=NCORES,
    )
    data = nc.dram_tensor("data", [2 * NPIX, 1], F32, kind="ExternalInput")
    offs = nc.dram_tensor("offs", [G, M], I32, kind="ExternalInput")
    onesd = nc.dram_tensor("onesd", [G, 1], F32, kind="ExternalInput")
    identd = nc.dram_tensor("identd", [G, G], BF16, kind="ExternalInput")
    rowsd = nc.dram_tensor("rowsd", [1, G], BF16, kind="ExternalInput")
    out = nc.dram_tensor("out", [G, M], F32, kind="ExternalOutput")

    with tile.TileContext(nc) as tc:
        _kernel_body(nc, tc, data, offs, onesd, identd, rowsd, out)
    nc.compile()
    return nc


def _kernel_body(nc, tc, data, offs, onesd, identd, rowsd, out):
    with (
        tc.tile_pool(name="sb", bufs=1) as sb,
        tc.tile_pool(name="ps", bufs=1, space="PSUM") as ps,
        tc.tile_pool(name="psr", bufs=2, space="PSUM") as psr,
    ):
        # ---- loads; the first gather is gated only by the first offs
        # DMA: split so a tiny 4-column load (shorter end-to-end) un-gates
        # the stream, the rest lands under the first gathers. HWDGE
        # first-use INDIRECT1D setup is only ~0.1us -- no warm-up needed.
        off = sb.tile([G, M], I32, tag="off")
        nc.sync.dma_start(out=off[:, 0:4], in_=offs.ap()[:, 0:4])
        nc.sync.dma_start(out=off[:, 4:M], in_=offs.ap()[:, 4:M])
        ident16 = sb.tile([G, G], BF16, tag="ident16")
        nc.scalar.dma_start(out=ident16[:], in_=identd.ap())
        ones16 = sb.tile([1, G], BF16, tag="ones16")
        nc.scalar.dma_start(out=ones16[:], in_=rowsd.ap())
        onescol = sb.tile([G, 1], F32, tag="onescol")
        nc.scalar.dma_start(out=onescol[:], in_=onesd.ap())

        # preload the Abs activation table while DMAs are in flight
        scrd = sb.tile([1, 1], F32, tag="scrd")
        nc.scalar.activation(out=scrd[:], in_=onescol[0:1, 0:1], func=AF.Abs)

        # ---- 16 gathers streaming on gpsimd ----
        dcol = sb.tile([G, M], F32, tag="dcol")
        for m in range(M):
            nc.gpsimd.indirect_dma_start(
                out=dcol[:, m : m + 1], out_offset=None, in_=data.ap(),
                in_offset=IndirectOffsetOnAxis(ap=off[:, m : m + 1], axis=0),
            )

        dv = dcol[:].rearrange("g (b t) -> g b t", b=BP, t=2)
        me = sb.tile([G, BP], BF16, tag="me")
        negme = sb.tile([G, BP], F32, tag="negme")
        fin = sb.tile([G, M], F32, tag="fin")   # cols 0:8 sq, 8:16 min

        dsub = sb.tile([G, BP], F32, tag="dsub")

        def push_image(b):
            bs = slice(b, b + 1)
            nc.vector.tensor_tensor(out=me[:, bs], in0=dv[:, b, 0:1],
                                    in1=dv[:, b, 1:2], op=ALU.add)
            nc.vector.tensor_tensor(out=dsub[:, bs], in0=dv[:, b, 0:1],
                                    in1=dv[:, b, 1:2], op=ALU.subtract)
            nc.vector.tensor_tensor(out=fin[:, bs], in0=dsub[:, bs],
                                    in1=dsub[:, bs], op=ALU.mult)
            nc.vector.tensor_scalar(out=negme[:, bs], in0=me[:, bs],
                                    scalar1=-0.5, scalar2=None, op0=ALU.mult)
            rowp = psr.tile([1, G], BF16, tag="rowp")
            nc.tensor.transpose(out=rowp[:], in_=me[:, bs], identity=ident16[:])
            merow = sb.tile([1, G], BF16, tag=f"merow{b % 2}")
            nc.vector.tensor_copy(out=merow[:], in_=rowp[:])
            Rp = psr.tile([G, G], F32, tag="Rp")
            nc.tensor.matmul(out=Rp[:], lhsT=ones16[:], rhs=merow[:],
                             start=True, stop=True)
            # ad lives in PSUM: the Abs write and the min read stay off
            # the SBUF ports that Q7 descriptor generation contends on
            ad = psr.tile([G, G], F32, tag="ad")
            nc.scalar.activation(out=ad[:], in_=Rp[:], func=AF.Abs,
                                 bias=negme[:, bs], scale=0.5)
            return ad

        ads = [None, None]
        for b in range(BP):
            # lag the DVE min by one image so the vector queue never
            # stalls waiting on this image's ABS
            if b >= 1:
                pb = b - 1
                nc.vector.tensor_scalar(
                    out=ads[pb % 2][:], in0=ads[pb % 2][:], scalar1=1.0,
                    scalar2=0.0, op0=ALU.min, op1=ALU.add,
                    accum_out=fin[:, BP + pb : BP + pb + 1],
                )
            ads[b % 2] = push_image(b)

        nc.vector.tensor_scalar(
            out=ads[(BP - 1) % 2][:], in0=ads[(BP - 1) % 2][:], scalar1=1.0,
            scalar2=0.0, op0=ALU.min, op1=ALU.add,
            accum_out=fin[:, M - 1 : M],
        )

        # ---- ship fin [128,16] back; the host all-reduce does the
        # column sums + the two scalar affine maps (cross-core
        # reduction is host-side regardless) ----
        nc.sync.dma_start(out=out.ap(), in_=fin[:])


_NC_CACHE = None


def _get_nc():
    global _NC_CACHE
    if _NC_CACHE is None:
        _NC_CACHE = _build_nc()
    return _NC_CACHE


def _consts():
    ones = np.ones((G, 1), dtype=np.float32)
    ident = np.eye(G, dtype=np.float32).astype(ml_dtypes.bfloat16)
    onesrow = np.ones((1, G), dtype=ml_dtypes.bfloat16)
    return ones, ident, onesrow


def make_in_maps(pred, target, match):
    pred = np.asarray(pred, dtype=np.float32).reshape(B, H * W)
    target = np.asarray(target, dtype=np.float32).reshape(B, H * W)
    match = np.asarray(match).astype(np.int64)
    ones, ident, onesrow = _consts()
    HW = H * W
    in_maps = []
    for k in range(NCORES):
        sl = slice(k * BP, (k + 1) * BP)
        data = np.concatenate(
            [pred[sl].reshape(-1), target[sl].reshape(-1)]
        ).reshape(2 * NPIX, 1)
        m = match[sl]  # [BP, G, 2, 2]
        offs = np.empty((G, M), dtype=np.int32)
        for b in range(BP):
            offs[:, 2 * b] = b * HW + m[b, :, 0, 0] * W + m[b, :, 0, 1]
            offs[:, 2 * b + 1] = NPIX + b * HW + m[b, :, 1, 0] * W + m[b, :, 1, 1]
        in_maps.append({
            "data": data,
            "offs": offs,
            "onesd": ones,
            "identd": ident,
            "rowsd": onesrow,
        })
    return in_maps


def kernel(pred, target, match, _trace=False):
    nc = _get_nc()
    in_maps = make_in_maps(pred, target, match)
    res = run_bass_kernel_spmd(nc, in_maps, core_ids=list(range(NCORES)), trace=_trace)
    total = np.zeros((M,), dtype=np.float64)
    for r in res.results:
        total += r["out"].astype(np.float64).sum(axis=0)
    pull = C_PULL * float(total[0:BP].sum())
    push = NCORES * BP * N * (N - 1) * C_PUSH - C_PUSH * float(
        total[BP:M].sum())
    out = (np.float32(pull), np.float32(push))
    if _trace:
        return out, res
    return out
